# revision 30
# baseline (speedup 1.0000x reference)
"""Fused transformer block (LN1 -> causal MHA -> residual -> LN2 -> FFN -> residual)
for Trainium2, distributed over 8 NeuronCores.

Sharding (v3, tensor-parallel attention per the head-split scheme):
  core c: batch b = c//4, head group g = c%4 (heads 4g..4g+3).
  Each core LNs the full sequence, projects K/V/Q for its 4 heads only,
  runs causal attention for those heads over all T=2048 queries with the
  upper-triangle key blocks skipped (uniform across cores, so SPMD holds),
  computes the Wo row-slice partial product, and ReduceScatters partials
  (bf16, per 512-query strip, overlapped with the next strip's attention)
  across the 4 cores of its batch. Each core then owns 512 tokens
  (4 scattered 128-blocks) for the residual + LN2 + FFN tail.
Matmuls run in bf16 with fp32 PSUM accumulation; LN/softmax math in fp32.
"""

import sys

import numpy as np

if "/opt/trn_rl_repo" not in sys.path:
    sys.path.insert(0, "/opt/trn_rl_repo")

import ml_dtypes

B, T, D = 2, 2048, 1024
H, HS = 16, 64
F = 4 * D
HG = 4            # heads per core
CH = HG * HS      # channels per core (256)
TQ = 512          # output tokens per core
NCORES = 8
EPS = 1e-5
NEG = -1e9

BF16 = ml_dtypes.bfloat16

DEBUG = False

_CACHE = {}


def _build(flags):
    """Build the Bass program (same for all cores). flags: (has_bo, has_b2)."""
    import concourse.bass as bass
    import concourse.mybir as mybir
    import concourse.tile as tile
    from concourse import bacc
    from concourse.bass import ts
    from concourse.masks import make_identity

    has_bo, has_b2 = flags
    f32 = mybir.dt.float32
    bf16 = mybir.dt.bfloat16
    Alu = mybir.AluOpType
    Act = mybir.ActivationFunctionType

    nc = bacc.Bacc("TRN2", target_bir_lowering=False, debug=False, num_devices=8)

    # ---- DRAM I/O ----
    x_kv = nc.dram_tensor("x_kv", [T, D], bf16, kind="ExternalInput").ap()
    x_q = nc.dram_tensor("x_q", [TQ, D], f32, kind="ExternalInput").ap()
    maskD = nc.dram_tensor("maskD", [128, 8 * TQ], bf16, kind="ExternalInput").ap()
    wq = nc.dram_tensor("wq", [D, CH], bf16, kind="ExternalInput").ap()
    wk = nc.dram_tensor("wk", [D, CH], bf16, kind="ExternalInput").ap()
    wv = nc.dram_tensor("wv", [D, CH], bf16, kind="ExternalInput").ap()
    wo = nc.dram_tensor("wo", [CH, D], bf16, kind="ExternalInput").ap()
    w1 = nc.dram_tensor("w1", [D, F], bf16, kind="ExternalInput").ap()
    w2 = nc.dram_tensor("w2", [F, D], bf16, kind="ExternalInput").ap()
    b1d = nc.dram_tensor("b1", [F], f32, kind="ExternalInput").ap()
    bod = nc.dram_tensor("bo", [D], f32, kind="ExternalInput").ap() if has_bo else None
    b2d = nc.dram_tensor("b2", [D], f32, kind="ExternalInput").ap() if has_b2 else None
    out = nc.dram_tensor("out", [TQ, D], f32, kind="ExternalOutput").ap()
    if DEBUG:
        dbg_h = nc.dram_tensor("dbg_h", [128, T], bf16, kind="ExternalOutput").ap()
        dbg_k = nc.dram_tensor("dbg_k", [128, T], bf16, kind="ExternalOutput").ap()
        dbg_q = nc.dram_tensor("dbg_q", [128, T], bf16, kind="ExternalOutput").ap()
        dbg_at = nc.dram_tensor("dbg_at", [128, T], bf16, kind="ExternalOutput").ap()
        dbg_pt = nc.dram_tensor("dbg_pt", [T, D], bf16, kind="ExternalOutput").ap()
        dbg_wr = nc.dram_tensor("dbg_wr", [TQ, D], bf16, kind="ExternalOutput").ap()
        dbg_x2 = nc.dram_tensor("dbg_x2", [TQ, D], f32, kind="ExternalOutput").ap()

    KT = T // 128      # 16 token tiles
    DC = D // 128      # 8 feature chunks of the model dim
    CC = CH // 128     # 2 channel chunks per core
    FC = F // 128      # 32 hidden chunks
    QS = TQ // 128     # 4 query subtiles per strip
    NS = T // TQ       # 4 query strips
    RGROUPS = [[0, 1, 2, 3], [4, 5, 6, 7]]

    with tile.TileContext(nc) as tc:
        with (
            tc.tile_pool(name="const", bufs=1) as cst,
            tc.tile_pool(name="actB", bufs=1) as actB,
            tc.tile_pool(name="dram", bufs=1, space="DRAM") as dram,
        ):
            # --- constants ---
            ident = cst.tile([128, 128], bf16)
            make_identity(nc, ident)
            eps_t = cst.tile([128, 1], f32)
            nc.gpsimd.memset(eps_t, EPS)
            b1_sb = cst.tile([128, FC], f32)
            nc.gpsimd.dma_start(out=b1_sb, in_=b1d.rearrange("(m p) -> p m", p=128))
            if has_bo:
                bo_b = cst.tile([128, D], f32)
                nc.gpsimd.dma_start(
                    out=bo_b,
                    in_=bass.AP(tensor=bod.tensor, offset=bod.offset,
                                ap=[[0, 128]] + list(bod.ap)))
            if has_b2:
                b2_b = cst.tile([128, D], f32)
                nc.gpsimd.dma_start(
                    out=b2_b,
                    in_=bass.AP(tensor=b2d.tensor, offset=b2d.offset,
                                ap=[[0, 128]] + list(b2d.ap)))
            mask_sb = cst.tile([128, 4, 2 * TQ], bf16)
            nc.gpsimd.dma_start(
                out=mask_sb, in_=maskD.rearrange("p (k q) -> p k q", q=2 * TQ))

            # --- persistent activations ---
            q_fm = [actB.tile([128, T], bf16, name=f"qfm{m}") for m in range(CC)]
            k_fm = [actB.tile([128, T], bf16, name=f"kfm{m}") for m in range(CC)]
            v_sb = [actB.tile([128, HG, HS + 1], bf16, name=f"vsb{t}")
                    for t in range(KT)]
            x2_sb = [actB.tile([128, D], f32, name=f"x2{i}") for i in range(QS)]
            h2_fm = [actB.tile([128, TQ], bf16, name=f"h2f{d}") for d in range(DC)]

            # DRAM bounce buffers for the per-strip ReduceScatter
            woD = [dram.tile([TQ, D], bf16, name=f"woD{s}") for s in range(NS)]
            woR = [dram.tile([128, D], bf16, name=f"woR{s}") for s in range(NS)]

            # ================= Phase 1: LN1 + transpose + V/K/Q ============
            with tc.tile_pool(name="hfmP", bufs=1) as hfmP:
              h_fm = [hfmP.tile([128, T], bf16, name=f"hfm{d}")
                      for d in range(DC)]
              with (
                tc.tile_pool(name="ph12", bufs=3) as ph12,
                tc.tile_pool(name="wP", bufs=1) as wP,
                tc.tile_pool(name="psV", bufs=3, space="PSUM") as psV,
              ):
                wvc = wP.tile([128, DC, CH], bf16, name="wvc")
                nc.scalar.dma_start(
                    out=wvc, in_=wv.rearrange("(k p) c -> p k c", p=128))
                wkc = wP.tile([128, DC, CH], bf16, name="wkc")
                nc.scalar.dma_start(
                    out=wkc, in_=wk.rearrange("(k p) c -> p k c", p=128))
                wqc = wP.tile([128, DC, CH], bf16, name="wqc")
                nc.scalar.dma_start(
                    out=wqc, in_=wq.rearrange("(k p) c -> p k c", p=128))
                for t in range(KT):
                    nc.gpsimd.memset(v_sb[t][:, :, HS:HS + 1], 1.0)
                sc = dram.tile([4, 128, 8], bf16, name="lnscr")
                # pipeline in strips of 512 tokens: stats -> mu/rstd rows ->
                # broadcast -> channel-major LN apply -> V/K/Q projections
                for n in range(NS):
                    mus = ph12.tile([128, 8], bf16, tag="mus", name="mus")
                    for tt in range(4):
                        t = 4 * n + tt
                        xt = ph12.tile([128, D], bf16, tag="xt", name="xt")
                        nc.sync.dma_start(out=xt, in_=x_kv[ts(t, 128), :])
                        xg = xt.rearrange("p (g f) -> p g f", f=512)
                        stats = ph12.tile([128, 2, 6], f32, tag="st", name="st")
                        for sg in range(2):
                            nc.vector.bn_stats(out=stats[:, sg, :],
                                               in_=xg[:, sg, :])
                        mv = ph12.tile([128, 2], f32, tag="mv", name="mv")
                        nc.vector.bn_aggr(out=mv, in_=stats)
                        rstd = ph12.tile([128, 1], f32, tag="rs", name="rs")
                        nc.scalar.activation(out=rstd, in_=mv[:, 1:2],
                                             func=Act.Sqrt, bias=eps_t,
                                             scale=1.0)
                        nc.vector.reciprocal(out=rstd, in_=rstd)
                        nc.vector.tensor_copy(out=mus[:, tt:tt + 1],
                                              in_=mv[:, 0:1])
                        nc.vector.tensor_copy(out=mus[:, 4 + tt:5 + tt],
                                              in_=rstd)
                    # [128, 4+4] -> [1, 512] mu and rstd rows via DRAM bounce
                    nc.sync.dma_start(out=sc[n], in_=mus)
                    mur = ph12.tile([1, TQ], bf16, tag="mur", name="mur")
                    rsr = ph12.tile([1, TQ], bf16, tag="rsr", name="rsr")
                    nc.sync.dma_start(
                        out=mur.rearrange("o (t p) -> o t p", p=128),
                        in_=bass.AP(tensor=sc.tensor,
                                    offset=sc.offset + n * 1024,
                                    ap=[[0, 1], [1, 4], [8, 128]]))
                    nc.sync.dma_start(
                        out=rsr.rearrange("o (t p) -> o t p", p=128),
                        in_=bass.AP(tensor=sc.tensor,
                                    offset=sc.offset + n * 1024 + 4,
                                    ap=[[0, 1], [1, 4], [8, 128]]))
                    mub = ph12.tile([128, TQ], bf16, tag="mub", name="mub")
                    rsb = ph12.tile([128, TQ], bf16, tag="rsb", name="rsb")
                    nc.gpsimd.partition_broadcast(mub, mur)
                    nc.gpsimd.partition_broadcast(rsb, rsr)
                    # channel-major LN apply (XBAR transpose of x from DRAM)
                    for d in range(DC):
                        xtd = ph12.tile([128, TQ], bf16, tag="xtd", name="xtd")
                        nc.sync.dma_start_transpose(
                            out=xtd, in_=x_kv[ts(n, TQ), ts(d, 128)])
                        hf = ph12.tile([128, TQ], bf16, tag="hf", name="hf")
                        nc.vector.tensor_tensor(out=hf, in0=xtd, in1=mub,
                                                op=Alu.subtract)
                        nc.vector.tensor_tensor(out=h_fm[d][:, ts(n, TQ)],
                                                in0=hf, in1=rsb, op=Alu.mult)
                    # V projection for this strip's 4 token tiles
                    for tt in range(4):
                        t = 4 * n + tt
                        pvv = psV.tile([128, CH], f32, tag="mm", name="psv")
                        for k in range(DC):
                            nc.tensor.matmul(pvv, h_fm[k][:, ts(t, 128)],
                                             wvc[:, k, :],
                                             start=(k == 0),
                                             stop=(k == DC - 1))
                        if t % 2 == 0:
                            nc.vector.tensor_copy(
                                out=v_sb[t][:, :, 0:HS],
                                in_=pvv.rearrange("p (h d) -> p h d", d=HS))
                        else:
                            nc.scalar.copy(
                                out=v_sb[t][:, :, 0:HS],
                                in_=pvv.rearrange("p (h d) -> p h d", d=HS))
                    # K and Q projections for this strip
                    for dst, wc in ((k_fm, wkc), (q_fm, wqc)):
                        for m in range(CC):
                            psk = psV.tile([128, 512], f32, tag="kq",
                                           name="psk")
                            for k in range(DC):
                                nc.tensor.matmul(
                                    psk, wc[:, k, ts(m, 128)],
                                    h_fm[k][:, ts(n, 512)],
                                    start=(k == 0), stop=(k == DC - 1))
                            if m % 2 == 0:
                                nc.vector.tensor_copy(
                                    out=dst[m][:, ts(n, 512)], in_=psk)
                            else:
                                nc.scalar.copy(
                                    out=dst[m][:, ts(n, 512)], in_=psk)
                if DEBUG:
                    nc.sync.dma_start(out=dbg_h, in_=h_fm[0])
                    nc.sync.dma_start(out=dbg_k, in_=k_fm[0])
                    nc.sync.dma_start(out=dbg_q, in_=q_fm[0])

            # ====== Phase 2: attention + Wo partials + ReduceScatter =======
            with (
                tc.tile_pool(name="ph4", bufs=7) as ph4,
                tc.tile_pool(name="smm", bufs=4) as smm,
                tc.tile_pool(name="atn", bufs=4) as atn,
                tc.tile_pool(name="ph5", bufs=3) as ph5,
                tc.tile_pool(name="psS", bufs=2, space="PSUM") as psS,
                tc.tile_pool(name="psAV", bufs=2, space="PSUM") as psAV,
                tc.tile_pool(name="psO", bufs=1, space="PSUM") as psO,
            ):
                wo_sb = []
                for k in range(CC):
                    wt = ph5.tile([128, D], bf16, tag=f"wo{k}", name=f"wo{k}")
                    nc.sync.dma_start(out=wt, in_=wo[ts(k, 128), :])
                    wo_sb.append(wt)

                strip_at = {}  # qs -> [attn tile per hp]

                def wo_block(qs):
                    # Wo partial for strip qs -> DRAM -> ReduceScatter
                    at = strip_at.pop(qs)
                    for ii in range(QS):
                        pso = psO.tile([128, D], f32, tag="o", name="pso")
                        for k in range(CC):
                            for n in range(2):
                                nc.tensor.matmul(
                                    pso[:, ts(n, 512)],
                                    at[k][:, ts(ii, 128)],
                                    wo_sb[k][:, ts(n, 512)],
                                    start=(k == 0), stop=(k == CC - 1))
                        wop = ph5.tile([128, D], bf16, tag="wop", name="wop")
                        nc.vector.tensor_copy(out=wop, in_=pso)
                        nc.sync.dma_start(out=woD[qs][ts(ii, 128), :], in_=wop)
                        if DEBUG:
                            nc.sync.dma_start(
                                out=dbg_pt[ts(4 * qs + ii, 128), :], in_=wop)
                    if DEBUG:
                        nc.sync.dma_start(out=dbg_at[:, ts(qs, TQ)], in_=at[0])
                    nc.gpsimd.collective_compute(
                        "ReduceScatter",
                        mybir.AluOpType.add,
                        replica_groups=RGROUPS,
                        ins=[woD[qs][:].opt()],
                        outs=[woR[qs][:].opt()],
                    )

                def emit_av(ent):
                    hp, qs, pav0, pav1, at_hp, kb, first, last, et = ent
                    for sub, pav in ((0, pav0), (1, pav1)):
                        nc.tensor.matmul(pav, v_sb[kb][:, 2 * hp + sub, :],
                                         et[:, ts(sub, TQ)],
                                         start=first, stop=last)
                    if not last:
                        return
                    for sub, pav in ((0, pav0), (1, pav1)):
                        ro = sub * HS
                        raw = smm.tile([HS + 1, TQ], f32, tag="raw",
                                       name="raw")
                        nc.vector.tensor_copy(out=raw, in_=pav)
                        recip = smm.tile([1, TQ], f32, tag="recip",
                                         name="recip")
                        nc.vector.reciprocal(out=recip, in_=raw[HS:HS + 1, :])
                        bcast = smm.tile([HS, TQ], f32, tag="bcast",
                                         name="bcast")
                        nc.gpsimd.partition_broadcast(bcast, recip)
                        nc.vector.tensor_tensor(
                            out=at_hp[ro:ro + HS, :],
                            in0=raw[0:HS, :], in1=bcast, op=Alu.mult)
                    if hp == 1:
                        wo_block(qs)

                LAG = 4
                pending = []
                for qs in range(NS):
                    nkb = 4 * qs + 4
                    strip_at[qs] = []
                    for hp in range(2):
                        at_hp = atn.tile([128, TQ], bf16, tag=f"at{hp}",
                                         name=f"at{hp}")
                        strip_at[qs].append(at_hp)
                        pav0 = psAV.tile([HS + 1, TQ], f32, tag="av",
                                         name="pav0")
                        pav1 = psAV.tile([HS + 1, TQ], f32, tag="av",
                                         name="pav1")
                        for kb in range(nkb):
                            pss = psS.tile([128, 2 * TQ], f32, tag="s",
                                           name="pss")
                            for sub in range(2):
                                ro = sub * HS
                                nc.tensor.matmul(
                                    pss[:, ts(sub, TQ)],
                                    k_fm[hp][ro:ro + HS, ts(kb, 128)],
                                    q_fm[hp][ro:ro + HS, ts(qs, TQ)],
                                    start=True, stop=True)
                            et = ph4.tile([128, 2 * TQ], bf16, tag="exp",
                                          name="et")
                            nc.scalar.activation(out=et, in_=pss, func=Act.Exp,
                                                 scale=0.125)
                            if kb >= 4 * qs:
                                i = kb - 4 * qs
                                nc.vector.tensor_tensor(
                                    out=et, in0=et, in1=mask_sb[:, i, :],
                                    op=Alu.mult)
                            pending.append(
                                (hp, qs, pav0, pav1, at_hp, kb, kb == 0,
                                 kb == nkb - 1, et))
                            if len(pending) > LAG:
                                emit_av(pending.pop(0))
                for ent in pending:
                    emit_av(ent)

            # ========= Phase 5+6: residual + LN2 + transpose ==============
            with (
                tc.tile_pool(name="ph6", bufs=3) as ph6,
                tc.tile_pool(name="psT2", bufs=4, space="PSUM") as psT2,
            ):
                for i in range(QS):
                    # on the scalar HWDGE queue: the woR read waits on the
                    # ReduceScatter, and must not head-of-line-block the sync
                    # queue (partial writes + FFN weight streams live there)
                    xq_sb = ph6.tile([128, D], f32, tag="xq", name="xq")
                    nc.scalar.dma_start(out=xq_sb, in_=x_q[ts(i, 128), :])
                    wor = ph6.tile([128, D], bf16, tag="wor", name="wor")
                    nc.scalar.dma_start(out=wor, in_=woR[i][:])
                    if DEBUG:
                        nc.sync.dma_start(out=dbg_wr[ts(i, 128), :], in_=wor)
                    nc.vector.tensor_tensor(
                        out=x2_sb[i], in0=xq_sb, in1=wor, op=Alu.add)
                    if has_bo:
                        nc.vector.tensor_tensor(
                            out=x2_sb[i], in0=x2_sb[i], in1=bo_b, op=Alu.add)
                    xg = x2_sb[i].rearrange("p (n f) -> p n f", f=512)
                    stats = ph6.tile([128, 2, 6], f32, tag="st", name="st6")
                    for sg in range(2):
                        nc.vector.bn_stats(out=stats[:, sg, :], in_=xg[:, sg, :])
                    mv = ph6.tile([128, 2], f32, tag="mv", name="mv6")
                    nc.vector.bn_aggr(out=mv, in_=stats)
                    rstd = ph6.tile([128, 1], f32, tag="rs", name="rs6")
                    nc.scalar.activation(out=rstd, in_=mv[:, 1:2], func=Act.Sqrt,
                                         bias=eps_t, scale=1.0)
                    nc.vector.reciprocal(out=rstd, in_=rstd)
                    if DEBUG:
                        nc.sync.dma_start(out=dbg_x2[ts(i, 128), :],
                                          in_=x2_sb[i])
                    h2t = ph6.tile([128, D], bf16, tag="h2t", name="h2t")
                    nc.vector.tensor_scalar(
                        out=h2t, in0=x2_sb[i], scalar1=mv[:, 0:1], scalar2=rstd,
                        op0=Alu.subtract, op1=Alu.mult)
                    for d in range(DC):
                        ps = psT2.tile([128, 128], bf16, tag="tr", name="tr2")
                        nc.tensor.transpose(ps, h2t[:, ts(d, 128)], ident)
                        nc.vector.tensor_copy(out=h2_fm[d][:, ts(i, 128)], in_=ps)

            # ================= Phase 7: FFN ===============================
            with tc.tile_pool(name="g1P", bufs=1) as g1P:
                g1 = [g1P.tile([128, TQ], bf16, name=f"g1t{m}") for m in range(FC)]
                with (
                    tc.tile_pool(name="ph7", bufs=3) as ph7,
                    tc.tile_pool(name="ph8", bufs=4) as ph8,
                    tc.tile_pool(name="psF", bufs=2, space="PSUM") as psF,
                    tc.tile_pool(name="ps8", bufs=1, space="PSUM") as ps8,
                ):
                    def ffn2_evac(psum2, n):
                        for i in range(QS):
                            ot = ph8.tile([128, 512], f32, tag="ot", name="ot")
                            nc.vector.tensor_tensor(
                                out=ot, in0=psum2[i],
                                in1=x2_sb[i][:, ts(n, 512)], op=Alu.add)
                            if has_b2:
                                nc.vector.tensor_tensor(
                                    out=ot, in0=ot, in1=b2_b[:, ts(n, 512)],
                                    op=Alu.add)
                            nc.sync.dma_start(out=out[ts(i, 128), ts(n, 512)],
                                              in_=ot)

                    # FFN1 interleaved with FFN2 n=0 half (W2 left columns)
                    psum2a = [ps8.tile([128, 512], f32, tag=f"p8_{j}",
                                       name=f"p8a{j}") for j in range(4)]
                    for m in range(FC):
                        wc = ph7.tile([128, DC, 128], bf16, tag="w1c", name="w1c")
                        nc.sync.dma_start(
                            out=wc,
                            in_=w1[:, ts(m, 128)].rearrange("(k p) c -> p k c", p=128))
                        ps = psF.tile([128, TQ], f32, tag="mm", name="psf")
                        for k in range(DC):
                            nc.tensor.matmul(ps, wc[:, k, :], h2_fm[k][:, 0:TQ],
                                             start=(k == 0), stop=(k == DC - 1))
                        nc.scalar.activation(out=g1[m], in_=ps, func=Act.Relu,
                                             bias=b1_sb[:, m:m + 1], scale=1.0)
                        w2c = ph8.tile([128, 512], bf16, tag="w2c", name="w2c")
                        nc.sync.dma_start(out=w2c, in_=w2[ts(m, 128), 0:512])
                        for i in range(QS):
                            nc.tensor.matmul(psum2a[i], g1[m][:, ts(i, 128)],
                                             w2c,
                                             start=(m == 0), stop=(m == FC - 1))
                    ffn2_evac(psum2a, 0)

                    # FFN2 n=1 half (W2 right columns)
                    psum2b = [ps8.tile([128, 512], f32, tag=f"p8_{j}",
                                       name=f"p8b{j}") for j in range(4)]
                    for m in range(FC):
                        w2c = ph8.tile([128, 512], bf16, tag="w2c", name="w2c")
                        nc.sync.dma_start(out=w2c, in_=w2[ts(m, 128), 512:1024])
                        for i in range(QS):
                            nc.tensor.matmul(psum2b[i], g1[m][:, ts(i, 128)],
                                             w2c,
                                             start=(m == 0), stop=(m == FC - 1))
                    ffn2_evac(psum2b, 1)

    nc.compile()
    return nc


def _token_rows(r):
    """Token rows owned by rank r after the per-strip ReduceScatter."""
    return np.concatenate(
        [np.arange(512 * qs + 128 * r, 512 * qs + 128 * r + 128)
         for qs in range(4)])


def _prep(inputs):
    """Host-side shard prep. Returns in_maps (one dict per core)."""
    x = np.asarray(inputs["x"], np.float32)
    ln1_g = np.asarray(inputs["ln1_g"], np.float32)
    ln1_b = np.asarray(inputs["ln1_b"], np.float32)
    ln2_g = np.asarray(inputs["ln2_g"], np.float32)
    ln2_b = np.asarray(inputs["ln2_b"], np.float32)
    assert np.all(ln1_b == 0.0) and np.all(ln2_b == 0.0), "ln biases must be 0"

    # fold ln gains into the consuming weight matrices
    wq = (ln1_g[:, None] * np.asarray(inputs["Wq"], np.float32)).astype(BF16)
    wk = (ln1_g[:, None] * np.asarray(inputs["Wk"], np.float32)).astype(BF16)
    wv = (ln1_g[:, None] * np.asarray(inputs["Wv"], np.float32)).astype(BF16)
    wo = np.asarray(inputs["Wo"], np.float32).astype(BF16)
    w1 = (ln2_g[:, None] * np.asarray(inputs["W1"], np.float32)).astype(BF16)
    w2 = np.asarray(inputs["W2"], np.float32).astype(BF16)
    b1 = np.ascontiguousarray(np.asarray(inputs["b1"], np.float32))
    bo = np.asarray(inputs["bo"], np.float32)
    b2 = np.asarray(inputs["b2"], np.float32)
    has_bo = bool(np.any(bo != 0.0))
    has_b2 = bool(np.any(b2 != 0.0))

    # multiplicative causal mask (applied to exp(scores)) for the 4 diagonal
    # key blocks of each 512-query strip, duplicated for the two heads
    # processed per score tile: mask[k, q] = 1 if q >= 128*i + k else 0
    kk = np.arange(128)[:, None]
    qq = np.arange(512)[None, :]
    blocks = []
    for i in range(4):
        mi = np.where(qq >= 128 * i + kk, np.float32(1.0), np.float32(0.0))
        blocks += [mi, mi]
    maskD = np.ascontiguousarray(np.concatenate(blocks, axis=1).astype(BF16))

    xb = x.astype(BF16)
    in_maps = []
    for c in range(NCORES):
        b, g = c // 4, c % 4
        rows = _token_rows(g)
        m = {
            "x_kv": np.ascontiguousarray(xb[b]),
            "x_q": np.ascontiguousarray(x[b][rows]),
            "maskD": maskD,
            "wq": np.ascontiguousarray(wq[:, g * CH:(g + 1) * CH]),
            "wk": np.ascontiguousarray(wk[:, g * CH:(g + 1) * CH]),
            "wv": np.ascontiguousarray(wv[:, g * CH:(g + 1) * CH]),
            "wo": np.ascontiguousarray(wo[g * CH:(g + 1) * CH, :]),
            "w1": w1, "w2": w2, "b1": b1,
        }
        if has_bo:
            m["bo"] = bo
        if has_b2:
            m["b2"] = b2
        in_maps.append(m)
    return in_maps, (has_bo, has_b2)


def _run(inputs, profile_dir=None):
    from concourse import bass_utils

    in_maps, flags = _prep(inputs)
    if flags not in _CACHE:
        _CACHE[flags] = _build(flags)
    nc = _CACHE[flags]

    if profile_dir is not None:
        from concourse import bass2jax
        from trn_agent_boot.trn_boot import _ntff_profile_via_ctypes
        hook = _ntff_profile_via_ctypes("/opt/axon/libaxon_pjrt.so")
        with hook(profile_dir, globals().get("PROFILE_CORES", [0])):
            results = bass2jax.run_bass_via_pjrt(nc, in_maps, n_cores=NCORES)
    else:
        res = bass_utils.run_bass_kernel_spmd(
            nc, in_maps, core_ids=list(range(NCORES))
        )
        results = res.results

    out = np.empty((B, T, D), np.float32)
    for c in range(NCORES):
        b, g = c // 4, c % 4
        out[b][_token_rows(g)] = results[c]["out"]
    return out


def kernel(**inputs) -> np.ndarray:
    return _run(inputs)


# revision 35
# speedup vs baseline: 1.0679x; 1.0679x over previous
"""Fused transformer block (LN1 -> causal MHA -> residual -> LN2 -> FFN -> residual)
for Trainium2, distributed over 8 NeuronCores.

Sharding (v3, tensor-parallel attention per the head-split scheme):
  core c: batch b = c//4, head group g = c%4 (heads 4g..4g+3).
  Each core LNs the full sequence, projects K/V/Q for its 4 heads only,
  runs causal attention for those heads over all T=2048 queries with the
  upper-triangle key blocks skipped (uniform across cores, so SPMD holds),
  computes the Wo row-slice partial product, and ReduceScatters partials
  (bf16, per 512-query strip, overlapped with the next strip's attention)
  across the 4 cores of its batch. Each core then owns 512 tokens
  (4 scattered 128-blocks) for the residual + LN2 + FFN tail.
Matmuls run in bf16 with fp32 PSUM accumulation; LN/softmax math in fp32.
"""

import sys

import numpy as np

if "/opt/trn_rl_repo" not in sys.path:
    sys.path.insert(0, "/opt/trn_rl_repo")

import ml_dtypes

B, T, D = 2, 2048, 1024
H, HS = 16, 64
F = 4 * D
HG = 4            # heads per core
CH = HG * HS      # channels per core (256)
TQ = 512          # output tokens per core
NCORES = 8
EPS = 1e-5
NEG = -1e9

BF16 = ml_dtypes.bfloat16

DEBUG = False

_CACHE = {}


def _build(flags):
    """Build the Bass program (same for all cores). flags: (has_bo, has_b2)."""
    import concourse.bass as bass
    import concourse.mybir as mybir
    import concourse.tile as tile
    from concourse import bacc
    from concourse.bass import ts
    from concourse.masks import make_identity

    has_bo, has_b2 = flags
    f32 = mybir.dt.float32
    bf16 = mybir.dt.bfloat16
    Alu = mybir.AluOpType
    Act = mybir.ActivationFunctionType

    nc = bacc.Bacc("TRN2", target_bir_lowering=False, debug=False, num_devices=8)

    # ---- DRAM I/O ----
    x_kv = nc.dram_tensor("x_kv", [T, D], bf16, kind="ExternalInput").ap()
    x_tr = nc.dram_tensor("x_tr", [D, T], bf16, kind="ExternalInput").ap()
    x_q = nc.dram_tensor("x_q", [TQ, D], f32, kind="ExternalInput").ap()
    maskD = nc.dram_tensor("maskD", [128, 8 * TQ], bf16, kind="ExternalInput").ap()
    wq = nc.dram_tensor("wq", [D, CH], bf16, kind="ExternalInput").ap()
    wk = nc.dram_tensor("wk", [D, CH], bf16, kind="ExternalInput").ap()
    wv = nc.dram_tensor("wv", [D, CH], bf16, kind="ExternalInput").ap()
    wo = nc.dram_tensor("wo", [CH, D], bf16, kind="ExternalInput").ap()
    w1 = nc.dram_tensor("w1", [D, F], bf16, kind="ExternalInput").ap()
    w2 = nc.dram_tensor("w2", [F, D], bf16, kind="ExternalInput").ap()
    b1d = nc.dram_tensor("b1", [F], f32, kind="ExternalInput").ap()
    bod = nc.dram_tensor("bo", [D], f32, kind="ExternalInput").ap() if has_bo else None
    b2d = nc.dram_tensor("b2", [D], f32, kind="ExternalInput").ap() if has_b2 else None
    out = nc.dram_tensor("out", [TQ, D], f32, kind="ExternalOutput").ap()
    if DEBUG:
        dbg_h = nc.dram_tensor("dbg_h", [128, T], bf16, kind="ExternalOutput").ap()
        dbg_k = nc.dram_tensor("dbg_k", [128, T], bf16, kind="ExternalOutput").ap()
        dbg_q = nc.dram_tensor("dbg_q", [128, T], bf16, kind="ExternalOutput").ap()
        dbg_at = nc.dram_tensor("dbg_at", [128, T], bf16, kind="ExternalOutput").ap()
        dbg_pt = nc.dram_tensor("dbg_pt", [T, D], bf16, kind="ExternalOutput").ap()
        dbg_wr = nc.dram_tensor("dbg_wr", [TQ, D], bf16, kind="ExternalOutput").ap()
        dbg_x2 = nc.dram_tensor("dbg_x2", [TQ, D], f32, kind="ExternalOutput").ap()

    KT = T // 128      # 16 token tiles
    DC = D // 128      # 8 feature chunks of the model dim
    CC = CH // 128     # 2 channel chunks per core
    FC = F // 128      # 32 hidden chunks
    QS = TQ // 128     # 4 query subtiles per strip
    NS = T // TQ       # 4 query strips
    RGROUPS = [[0, 1, 2, 3], [4, 5, 6, 7]]

    with tile.TileContext(nc) as tc:
        with (
            tc.tile_pool(name="const", bufs=1) as cst,
            tc.tile_pool(name="actB", bufs=1) as actB,
            tc.tile_pool(name="dram", bufs=1, space="DRAM") as dram,
        ):
            # --- constants ---
            ident = cst.tile([128, 128], bf16)
            make_identity(nc, ident)
            eps_t = cst.tile([128, 1], f32)
            nc.gpsimd.memset(eps_t, EPS)
            b1_sb = cst.tile([128, FC], f32)
            nc.gpsimd.dma_start(out=b1_sb, in_=b1d.rearrange("(m p) -> p m", p=128))
            if has_bo:
                bo_b = cst.tile([128, D], f32)
                nc.gpsimd.dma_start(
                    out=bo_b,
                    in_=bass.AP(tensor=bod.tensor, offset=bod.offset,
                                ap=[[0, 128]] + list(bod.ap)))
            if has_b2:
                b2_b = cst.tile([128, D], f32)
                nc.gpsimd.dma_start(
                    out=b2_b,
                    in_=bass.AP(tensor=b2d.tensor, offset=b2d.offset,
                                ap=[[0, 128]] + list(b2d.ap)))
            mask_sb = cst.tile([128, 4, 2 * TQ], bf16)
            nc.gpsimd.dma_start(
                out=mask_sb, in_=maskD.rearrange("p (k q) -> p k q", q=2 * TQ))

            # --- persistent activations ---
            q_fm = [actB.tile([128, T], bf16, name=f"qfm{m}") for m in range(CC)]
            k_fm = [actB.tile([128, T], bf16, name=f"kfm{m}") for m in range(CC)]
            v_sb = [actB.tile([128, HG, HS + 1], bf16, name=f"vsb{t}")
                    for t in range(KT)]
            x2_sb = [actB.tile([128, D], f32, name=f"x2{i}") for i in range(QS)]
            h2_fm = [actB.tile([128, TQ], bf16, name=f"h2f{d}") for d in range(DC)]

            # DRAM bounce buffers for the per-strip ReduceScatter
            woD = [dram.tile([TQ, D], bf16, name=f"woD{s}") for s in range(NS)]
            woR = [dram.tile([128, D], bf16, name=f"woR{s}") for s in range(NS)]

            # ================= Phase 1: LN1 + transpose + V/K/Q ============
            with tc.tile_pool(name="hfmP", bufs=1) as hfmP:
              h_fm = [hfmP.tile([128, T], bf16, name=f"hfm{d}")
                      for d in range(DC)]
              with (
                tc.tile_pool(name="ph12", bufs=3) as ph12,
                tc.tile_pool(name="wP", bufs=1) as wP,
                tc.tile_pool(name="psV", bufs=3, space="PSUM") as psV,
              ):
                wvc = wP.tile([128, DC, CH], bf16, name="wvc")
                nc.scalar.dma_start(
                    out=wvc, in_=wv.rearrange("(k p) c -> p k c", p=128))
                wkc = wP.tile([128, DC, CH], bf16, name="wkc")
                nc.scalar.dma_start(
                    out=wkc, in_=wk.rearrange("(k p) c -> p k c", p=128))
                wqc = wP.tile([128, DC, CH], bf16, name="wqc")
                nc.scalar.dma_start(
                    out=wqc, in_=wq.rearrange("(k p) c -> p k c", p=128))
                for t in range(KT):
                    nc.gpsimd.memset(v_sb[t][:, :, HS:HS + 1], 1.0)
                sc = dram.tile([4, 128, 8], bf16, name="lnscr")
                # pipeline in strips of 512 tokens: stats -> mu/rstd rows ->
                # broadcast -> channel-major LN apply -> V/K/Q projections
                for n in range(NS):
                    mus = ph12.tile([128, 8], bf16, tag="mus", name="mus")
                    for tt in range(4):
                        t = 4 * n + tt
                        xt = ph12.tile([128, D], bf16, tag="xt", name="xt")
                        nc.sync.dma_start(out=xt, in_=x_kv[ts(t, 128), :])
                        xg = xt.rearrange("p (g f) -> p g f", f=512)
                        stats = ph12.tile([128, 2, 6], f32, tag="st", name="st")
                        for sg in range(2):
                            nc.vector.bn_stats(out=stats[:, sg, :],
                                               in_=xg[:, sg, :])
                        mv = ph12.tile([128, 2], f32, tag="mv", name="mv")
                        nc.vector.bn_aggr(out=mv, in_=stats)
                        rstd = ph12.tile([128, 1], f32, tag="rs", name="rs")
                        nc.scalar.activation(out=rstd, in_=mv[:, 1:2],
                                             func=Act.Sqrt, bias=eps_t,
                                             scale=1.0)
                        nc.vector.reciprocal(out=rstd, in_=rstd)
                        nc.vector.tensor_copy(out=mus[:, tt:tt + 1],
                                              in_=mv[:, 0:1])
                        nc.vector.tensor_copy(out=mus[:, 4 + tt:5 + tt],
                                              in_=rstd)
                    # [128, 4+4] -> [1, 512] mu and rstd rows via DRAM bounce
                    nc.sync.dma_start(out=sc[n], in_=mus)
                    mur = ph12.tile([1, TQ], bf16, tag="mur", name="mur")
                    rsr = ph12.tile([1, TQ], bf16, tag="rsr", name="rsr")
                    nc.sync.dma_start(
                        out=mur.rearrange("o (t p) -> o t p", p=128),
                        in_=bass.AP(tensor=sc.tensor,
                                    offset=sc.offset + n * 1024,
                                    ap=[[0, 1], [1, 4], [8, 128]]))
                    nc.sync.dma_start(
                        out=rsr.rearrange("o (t p) -> o t p", p=128),
                        in_=bass.AP(tensor=sc.tensor,
                                    offset=sc.offset + n * 1024 + 4,
                                    ap=[[0, 1], [1, 4], [8, 128]]))
                    mub = ph12.tile([128, TQ], bf16, tag="mub", name="mub")
                    rsb = ph12.tile([128, TQ], bf16, tag="rsb", name="rsb")
                    nc.gpsimd.partition_broadcast(mub, mur)
                    nc.gpsimd.partition_broadcast(rsb, rsr)
                    # channel-major LN apply (x transposed on the host)
                    for d in range(DC):
                        xtd = ph12.tile([128, TQ], bf16, tag="xtd", name="xtd")
                        nc.sync.dma_start(
                            out=xtd, in_=x_tr[ts(d, 128), ts(n, TQ)])
                        hf = ph12.tile([128, TQ], bf16, tag="hf", name="hf")
                        nc.vector.tensor_tensor(out=hf, in0=xtd, in1=mub,
                                                op=Alu.subtract)
                        nc.vector.tensor_tensor(out=h_fm[d][:, ts(n, TQ)],
                                                in0=hf, in1=rsb, op=Alu.mult)
                    # V projection for this strip's 4 token tiles
                    for tt in range(4):
                        t = 4 * n + tt
                        pvv = psV.tile([128, CH], f32, tag="mm", name="psv")
                        for k in range(DC):
                            nc.tensor.matmul(pvv, h_fm[k][:, ts(t, 128)],
                                             wvc[:, k, :],
                                             start=(k == 0),
                                             stop=(k == DC - 1))
                        if t % 2 == 0:
                            nc.vector.tensor_copy(
                                out=v_sb[t][:, :, 0:HS],
                                in_=pvv.rearrange("p (h d) -> p h d", d=HS))
                        else:
                            nc.scalar.copy(
                                out=v_sb[t][:, :, 0:HS],
                                in_=pvv.rearrange("p (h d) -> p h d", d=HS))
                    # K and Q projections for this strip
                    for dst, wc in ((k_fm, wkc), (q_fm, wqc)):
                        for m in range(CC):
                            psk = psV.tile([128, 512], f32, tag="kq",
                                           name="psk")
                            for k in range(DC):
                                nc.tensor.matmul(
                                    psk, wc[:, k, ts(m, 128)],
                                    h_fm[k][:, ts(n, 512)],
                                    start=(k == 0), stop=(k == DC - 1))
                            if m % 2 == 0:
                                nc.vector.tensor_copy(
                                    out=dst[m][:, ts(n, 512)], in_=psk)
                            else:
                                nc.scalar.copy(
                                    out=dst[m][:, ts(n, 512)], in_=psk)
                if DEBUG:
                    nc.sync.dma_start(out=dbg_h, in_=h_fm[0])
                    nc.sync.dma_start(out=dbg_k, in_=k_fm[0])
                    nc.sync.dma_start(out=dbg_q, in_=q_fm[0])

            # ====== Phase 2: attention + Wo partials + ReduceScatter =======
            with (
                tc.tile_pool(name="ph4", bufs=7) as ph4,
                tc.tile_pool(name="smm", bufs=4) as smm,
                tc.tile_pool(name="atn", bufs=4) as atn,
                tc.tile_pool(name="ph5", bufs=3) as ph5,
                tc.tile_pool(name="psS", bufs=2, space="PSUM") as psS,
                tc.tile_pool(name="psAV", bufs=2, space="PSUM") as psAV,
                tc.tile_pool(name="psO", bufs=1, space="PSUM") as psO,
            ):
                wo_sb = []
                for k in range(CC):
                    wt = ph5.tile([128, D], bf16, tag=f"wo{k}", name=f"wo{k}")
                    nc.sync.dma_start(out=wt, in_=wo[ts(k, 128), :])
                    wo_sb.append(wt)

                strip_at = {}  # qs -> [attn tile per hp]

                def wo_block(qs):
                    # Wo partial for strip qs -> DRAM -> ReduceScatter
                    at = strip_at.pop(qs)
                    for ii in range(QS):
                        pso = psO.tile([128, D], f32, tag="o", name="pso")
                        for k in range(CC):
                            for n in range(2):
                                nc.tensor.matmul(
                                    pso[:, ts(n, 512)],
                                    at[k][:, ts(ii, 128)],
                                    wo_sb[k][:, ts(n, 512)],
                                    start=(k == 0), stop=(k == CC - 1))
                        wop = ph5.tile([128, D], bf16, tag="wop", name="wop")
                        nc.vector.tensor_copy(out=wop, in_=pso)
                        nc.sync.dma_start(out=woD[qs][ts(ii, 128), :], in_=wop)
                        if DEBUG:
                            nc.sync.dma_start(
                                out=dbg_pt[ts(4 * qs + ii, 128), :], in_=wop)
                    if DEBUG:
                        nc.sync.dma_start(out=dbg_at[:, ts(qs, TQ)], in_=at[0])
                    nc.gpsimd.collective_compute(
                        "ReduceScatter",
                        mybir.AluOpType.add,
                        replica_groups=RGROUPS,
                        ins=[woD[qs][:].opt()],
                        outs=[woR[qs][:].opt()],
                    )

                def emit_av(ent):
                    hp, qs, pav0, pav1, at_hp, kb, first, last, et = ent
                    for sub, pav in ((0, pav0), (1, pav1)):
                        nc.tensor.matmul(pav, v_sb[kb][:, 2 * hp + sub, :],
                                         et[:, ts(sub, TQ)],
                                         start=first, stop=last)
                    if not last:
                        return
                    for sub, pav in ((0, pav0), (1, pav1)):
                        ro = sub * HS
                        raw = smm.tile([HS + 1, TQ], f32, tag="raw",
                                       name="raw")
                        nc.vector.tensor_copy(out=raw, in_=pav)
                        recip = smm.tile([1, TQ], f32, tag="recip",
                                         name="recip")
                        nc.vector.reciprocal(out=recip, in_=raw[HS:HS + 1, :])
                        bcast = smm.tile([HS, TQ], f32, tag="bcast",
                                         name="bcast")
                        nc.gpsimd.partition_broadcast(bcast, recip)
                        nc.vector.tensor_tensor(
                            out=at_hp[ro:ro + HS, :],
                            in0=raw[0:HS, :], in1=bcast, op=Alu.mult)
                    if hp == 1:
                        wo_block(qs)

                LAG = 4
                pending = []
                for qs in range(NS):
                    nkb = 4 * qs + 4
                    strip_at[qs] = []
                    for hp in range(2):
                        at_hp = atn.tile([128, TQ], bf16, tag=f"at{hp}",
                                         name=f"at{hp}")
                        strip_at[qs].append(at_hp)
                        pav0 = psAV.tile([HS + 1, TQ], f32, tag="av",
                                         name="pav0")
                        pav1 = psAV.tile([HS + 1, TQ], f32, tag="av",
                                         name="pav1")
                        for kb in range(nkb):
                            pss = psS.tile([128, 2 * TQ], f32, tag="s",
                                           name="pss")
                            for sub in range(2):
                                ro = sub * HS
                                nc.tensor.matmul(
                                    pss[:, ts(sub, TQ)],
                                    k_fm[hp][ro:ro + HS, ts(kb, 128)],
                                    q_fm[hp][ro:ro + HS, ts(qs, TQ)],
                                    start=True, stop=True)
                            et = ph4.tile([128, 2 * TQ], bf16, tag="exp",
                                          name="et")
                            nc.scalar.activation(out=et, in_=pss, func=Act.Exp,
                                                 scale=0.125)
                            if kb >= 4 * qs:
                                i = kb - 4 * qs
                                nc.vector.tensor_tensor(
                                    out=et, in0=et, in1=mask_sb[:, i, :],
                                    op=Alu.mult)
                            pending.append(
                                (hp, qs, pav0, pav1, at_hp, kb, kb == 0,
                                 kb == nkb - 1, et))
                            if len(pending) > LAG:
                                emit_av(pending.pop(0))
                for ent in pending:
                    emit_av(ent)

            # ========= Phase 5+6: residual + LN2 + transpose ==============
            with (
                tc.tile_pool(name="ph6", bufs=3) as ph6,
                tc.tile_pool(name="psT2", bufs=4, space="PSUM") as psT2,
            ):
                for i in range(QS):
                    # on the scalar HWDGE queue: the woR read waits on the
                    # ReduceScatter, and must not head-of-line-block the sync
                    # queue (partial writes + FFN weight streams live there)
                    xq_sb = ph6.tile([128, D], f32, tag="xq", name="xq")
                    nc.scalar.dma_start(out=xq_sb, in_=x_q[ts(i, 128), :])
                    wor = ph6.tile([128, D], bf16, tag="wor", name="wor")
                    nc.scalar.dma_start(out=wor, in_=woR[i][:])
                    if DEBUG:
                        nc.sync.dma_start(out=dbg_wr[ts(i, 128), :], in_=wor)
                    nc.vector.tensor_tensor(
                        out=x2_sb[i], in0=xq_sb, in1=wor, op=Alu.add)
                    if has_bo:
                        nc.vector.tensor_tensor(
                            out=x2_sb[i], in0=x2_sb[i], in1=bo_b, op=Alu.add)
                    xg = x2_sb[i].rearrange("p (n f) -> p n f", f=512)
                    stats = ph6.tile([128, 2, 6], f32, tag="st", name="st6")
                    for sg in range(2):
                        nc.vector.bn_stats(out=stats[:, sg, :], in_=xg[:, sg, :])
                    mv = ph6.tile([128, 2], f32, tag="mv", name="mv6")
                    nc.vector.bn_aggr(out=mv, in_=stats)
                    rstd = ph6.tile([128, 1], f32, tag="rs", name="rs6")
                    nc.scalar.activation(out=rstd, in_=mv[:, 1:2], func=Act.Sqrt,
                                         bias=eps_t, scale=1.0)
                    nc.vector.reciprocal(out=rstd, in_=rstd)
                    if DEBUG:
                        nc.sync.dma_start(out=dbg_x2[ts(i, 128), :],
                                          in_=x2_sb[i])
                    h2t = ph6.tile([128, D], bf16, tag="h2t", name="h2t")
                    nc.vector.tensor_scalar(
                        out=h2t, in0=x2_sb[i], scalar1=mv[:, 0:1], scalar2=rstd,
                        op0=Alu.subtract, op1=Alu.mult)
                    for d in range(DC):
                        ps = psT2.tile([128, 128], bf16, tag="tr", name="tr2")
                        nc.tensor.transpose(ps, h2t[:, ts(d, 128)], ident)
                        nc.vector.tensor_copy(out=h2_fm[d][:, ts(i, 128)], in_=ps)

            # ================= Phase 7: FFN ===============================
            with tc.tile_pool(name="g1P", bufs=1) as g1P:
                g1 = [g1P.tile([128, TQ], bf16, name=f"g1t{m}") for m in range(FC)]
                # FFN1: dense m-sweep, W1 streamed once
                with (
                    tc.tile_pool(name="ph7", bufs=3) as ph7,
                    tc.tile_pool(name="psF", bufs=2, space="PSUM") as psF,
                ):
                    for m in range(FC):
                        wc = ph7.tile([128, DC, 128], bf16, tag="w1c", name="w1c")
                        nc.sync.dma_start(
                            out=wc,
                            in_=w1[:, ts(m, 128)].rearrange("(k p) c -> p k c", p=128))
                        ps = psF.tile([128, TQ], f32, tag="mm", name="psf")
                        for k in range(DC):
                            nc.tensor.matmul(ps, wc[:, k, :], h2_fm[k][:, 0:TQ],
                                             start=(k == 0), stop=(k == DC - 1))
                        nc.scalar.activation(out=g1[m], in_=ps, func=Act.Relu,
                                             bias=b1_sb[:, m:m + 1], scale=1.0)
                # FFN2: dense m-sweep over all 8 PSUM banks, W2 streamed once
                with (
                    tc.tile_pool(name="ph8", bufs=4) as ph8,
                    tc.tile_pool(name="ps8", bufs=1, space="PSUM") as ps8,
                ):
                    psum2 = [ps8.tile([128, 512], f32, tag=f"p8_{j}",
                                      name=f"p8_{j}") for j in range(8)]
                    for m in range(FC):
                        w2c = ph8.tile([128, D], bf16, tag="w2c", name="w2c")
                        nc.sync.dma_start(out=w2c, in_=w2[ts(m, 128), :])
                        for i in range(QS):
                            for n_ in range(2):
                                nc.tensor.matmul(
                                    psum2[i * 2 + n_], g1[m][:, ts(i, 128)],
                                    w2c[:, ts(n_, 512)],
                                    start=(m == 0), stop=(m == FC - 1))
                    for i in range(QS):
                        ot = ph8.tile([128, D], f32, tag="ot", name="ot")
                        for n_ in range(2):
                            nc.vector.tensor_tensor(
                                out=ot[:, ts(n_, 512)], in0=psum2[i * 2 + n_],
                                in1=x2_sb[i][:, ts(n_, 512)], op=Alu.add)
                        if has_b2:
                            nc.vector.tensor_tensor(
                                out=ot, in0=ot, in1=b2_b, op=Alu.add)
                        nc.sync.dma_start(out=out[ts(i, 128), :], in_=ot)

    nc.compile()
    return nc


def _token_rows(r):
    """Token rows owned by rank r after the per-strip ReduceScatter."""
    return np.concatenate(
        [np.arange(512 * qs + 128 * r, 512 * qs + 128 * r + 128)
         for qs in range(4)])


def _prep(inputs):
    """Host-side shard prep. Returns in_maps (one dict per core)."""
    x = np.asarray(inputs["x"], np.float32)
    ln1_g = np.asarray(inputs["ln1_g"], np.float32)
    ln1_b = np.asarray(inputs["ln1_b"], np.float32)
    ln2_g = np.asarray(inputs["ln2_g"], np.float32)
    ln2_b = np.asarray(inputs["ln2_b"], np.float32)
    assert np.all(ln1_b == 0.0) and np.all(ln2_b == 0.0), "ln biases must be 0"

    # fold ln gains into the consuming weight matrices
    wq = (ln1_g[:, None] * np.asarray(inputs["Wq"], np.float32)).astype(BF16)
    wk = (ln1_g[:, None] * np.asarray(inputs["Wk"], np.float32)).astype(BF16)
    wv = (ln1_g[:, None] * np.asarray(inputs["Wv"], np.float32)).astype(BF16)
    wo = np.asarray(inputs["Wo"], np.float32).astype(BF16)
    w1 = (ln2_g[:, None] * np.asarray(inputs["W1"], np.float32)).astype(BF16)
    w2 = np.asarray(inputs["W2"], np.float32).astype(BF16)
    b1 = np.ascontiguousarray(np.asarray(inputs["b1"], np.float32))
    bo = np.asarray(inputs["bo"], np.float32)
    b2 = np.asarray(inputs["b2"], np.float32)
    has_bo = bool(np.any(bo != 0.0))
    has_b2 = bool(np.any(b2 != 0.0))

    # multiplicative causal mask (applied to exp(scores)) for the 4 diagonal
    # key blocks of each 512-query strip, duplicated for the two heads
    # processed per score tile: mask[k, q] = 1 if q >= 128*i + k else 0
    kk = np.arange(128)[:, None]
    qq = np.arange(512)[None, :]
    blocks = []
    for i in range(4):
        mi = np.where(qq >= 128 * i + kk, np.float32(1.0), np.float32(0.0))
        blocks += [mi, mi]
    maskD = np.ascontiguousarray(np.concatenate(blocks, axis=1).astype(BF16))

    xb = x.astype(BF16)
    in_maps = []
    for c in range(NCORES):
        b, g = c // 4, c % 4
        rows = _token_rows(g)
        m = {
            "x_kv": np.ascontiguousarray(xb[b]),
            "x_tr": np.ascontiguousarray(xb[b].T),
            "x_q": np.ascontiguousarray(x[b][rows]),
            "maskD": maskD,
            "wq": np.ascontiguousarray(wq[:, g * CH:(g + 1) * CH]),
            "wk": np.ascontiguousarray(wk[:, g * CH:(g + 1) * CH]),
            "wv": np.ascontiguousarray(wv[:, g * CH:(g + 1) * CH]),
            "wo": np.ascontiguousarray(wo[g * CH:(g + 1) * CH, :]),
            "w1": w1, "w2": w2, "b1": b1,
        }
        if has_bo:
            m["bo"] = bo
        if has_b2:
            m["b2"] = b2
        in_maps.append(m)
    return in_maps, (has_bo, has_b2)


def _run(inputs, profile_dir=None):
    from concourse import bass_utils

    in_maps, flags = _prep(inputs)
    if flags not in _CACHE:
        _CACHE[flags] = _build(flags)
    nc = _CACHE[flags]

    if profile_dir is not None:
        from concourse import bass2jax
        from trn_agent_boot.trn_boot import _ntff_profile_via_ctypes
        hook = _ntff_profile_via_ctypes("/opt/axon/libaxon_pjrt.so")
        with hook(profile_dir, globals().get("PROFILE_CORES", [0])):
            results = bass2jax.run_bass_via_pjrt(nc, in_maps, n_cores=NCORES)
    else:
        res = bass_utils.run_bass_kernel_spmd(
            nc, in_maps, core_ids=list(range(NCORES))
        )
        results = res.results

    out = np.empty((B, T, D), np.float32)
    for c in range(NCORES):
        b, g = c // 4, c % 4
        out[b][_token_rows(g)] = results[c]["out"]
    return out


def kernel(**inputs) -> np.ndarray:
    return _run(inputs)
